# revision 7
# baseline (speedup 1.0000x reference)
"""Trainium2 Bass kernel for nn_BackboneModel (backbone frame rebuild).

The reference scatters rows into a padded [B, L, 14, 3] block, builds
Gram-Schmidt rigid frames from (N, CA, C), places ideal N/CA/C/O atoms,
and gathers the valid rows back.  Scatter followed by gather at the same
(batch_id, pos) indices is an identity permutation over the valid rows,
so the whole model is a pure per-row function of X[i]:

    e1 = normalize(C - CA)                      (normalize: v * rsqrt(|v|^2 + eps^2))
    e2 = normalize((N - CA) - ((N - CA).e1) e1)
    out[0] = -0.525*e1 + 1.363*e2 + CA          (N)
    out[1] = CA                                 (CA, passthrough)
    out[2] =  1.526*e1            + CA          (C)
    out[3] =  2.153*e1 - 1.062*e2 + CA          (O)
    out[4:14] = X[4:14]                         (passthrough)

Only atoms 0..2 (N, CA, C) feed any arithmetic, and only atoms 0, 2, 3
get new values.  The device reads a packed [rows, 9] f32 block (N, CA,
C) and writes a packed atom-major [3, rows, 3] bf16 block (outN, outC,
outO); CA and atoms 4..13 are stitched in on the host during unshard
(pure data movement).  Correctness gate is rel-L2 < 2e-2; bf16 rounding
of final coordinates contributes ~1e-3.

Numerics: the Gram-Schmidt rejection w = v - (v.e1)e1 amplifies error in
the projection scalar g by ~|v|/|w|, so g = (v.d1)/|d1|^2 uses the DVE
reciprocal-approx (18 bits) and w is formed in f32.  The unit scalings
rs1/rs2 only scale outputs, so the ACT table Sqrt is fine there.  The
reference's +eps^2 regularizer is dropped (inputs are randn; |d1|^2 is
never near eps^2 = 1e-6, relative effect ~1e-8).

Sharding: data-parallel, 8 equal contiguous row chunks of 98304 rows.
"""

import numpy as np

N_CORES = 8
N_TOTAL = 786432
N_CORE = N_TOTAL // N_CORES      # 98304 rows per core
P = 128                          # SBUF partitions
ROWS_PER_PART = N_CORE // P      # 768 rows per partition per core
TILE_SIZES = [64, 224, 256, 224]
CIN = 9                          # N, CA, C xyz (f32)
EPS2 = 1e-6                      # unused on device; kept for reference

_NC = None


def _build_nc():
    import concourse.bacc as bacc
    import concourse.tile as tile
    from concourse import mybir

    f32 = mybir.dt.float32
    bf16 = mybir.dt.bfloat16
    MUL = mybir.AluOpType.mult
    ADD = mybir.AluOpType.add
    AX = mybir.AxisListType.X
    SQRT = mybir.ActivationFunctionType.Sqrt
    SQUARE = mybir.ActivationFunctionType.Square
    COPY = mybir.ActivationFunctionType.Copy

    nc = bacc.Bacc()
    X = nc.declare_dram_parameter("X", [N_CORE, CIN], f32, isOutput=False)
    Y = nc.declare_dram_parameter("Y", [3, N_CORE, 3], bf16, isOutput=True)

    def bcast(s, r):  # [P, r] per-row scalar -> [P, r, 3]
        return s[:, :, None].broadcast_to([P, r, 3])

    with tile.TileContext(nc) as tc:
        with tc.tile_pool(name="io", bufs=4) as io, \
             tc.tile_pool(name="v3", bufs=3) as v3, \
             tc.tile_pool(name="sc", bufs=3) as sc:

            def head(row_off, R):
                st = {"R": R, "off": row_off}
                T = st["T"] = io.tile([P, R, CIN], f32, tag="T", name="T")
                nc.sync.dma_start(
                    out=T,
                    in_=X[row_off:row_off + P * R, :].rearrange(
                        "(p r) c -> p r c", p=P))
                Na = T[:, :, 0:3]
                CAa = T[:, :, 3:6]
                Ca = T[:, :, 6:9]

                D1 = v3.tile([P, R, 3], f32, tag="d1", name="d1")
                V = v3.tile([P, R, 3], f32, tag="v", name="v")
                QQ = v3.tile([P, R, 6], f32, tag="qq", name="qq")
                T1 = QQ[:, :, 0:3]
                W = v3.tile([P, R, 3], f32, tag="w", name="w")
                SQ2 = QQ[:, :, 3:6]
                CAb = st["CAb"] = v3.tile([P, R, 3], bf16, tag="cab",
                                          name="cab")
                D1b = st["D1b"] = v3.tile([P, R, 3], bf16, tag="d1b",
                                          name="d1b")
                Wb = st["Wb"] = v3.tile([P, R, 3], bf16, tag="wb", name="wb")
                RS1d = st["RS1d"] = v3.tile([P, R, 3], bf16, tag="rs1d",
                                            name="rs1d")
                RS2d = st["RS2d"] = v3.tile([P, R, 3], bf16, tag="rs2d",
                                            name="rs2d")
                SD = sc.tile([P, R, 2], f32, tag="sd", name="sd")
                IS1 = sc.tile([P, R], f32, tag="is1", name="is1")
                RS1 = sc.tile([P, R], f32, tag="rs1", name="rs1")
                G = sc.tile([P, R], f32, tag="g", name="g")
                S2 = sc.tile([P, R], f32, tag="s2", name="s2")
                IS2 = sc.tile([P, R], f32, tag="is2", name="is2")
                RS2 = sc.tile([P, R], f32, tag="rs2", name="rs2")

                # d1 = C - CA, v = N - CA  (Pool engine)
                nc.gpsimd.tensor_sub(D1, Ca, CAa)
                nc.gpsimd.tensor_sub(V, Na, CAa)
                # bf16 shadows for the 2x tail (ACT)
                nc.scalar.activation(out=CAb, in_=CAa, func=COPY)
                nc.scalar.activation(out=D1b, in_=D1, func=COPY)

                # fused |d1|^2 and v.d1: QQ = [d1^2 | v*d1], one reduce
                nc.scalar.activation(out=QQ[:, :, 0:3], in_=D1, func=SQUARE)
                nc.vector.tensor_mul(QQ[:, :, 3:6], V, D1)
                nc.vector.reduce_sum(
                    out=SD, in_=QQ.rearrange("p r (a c) -> p r a c", a=2),
                    axis=AX)
                nc.vector.reciprocal_approx_fast(out=IS1, in_=SD[:, :, 0])
                nc.scalar.activation(out=RS1, in_=IS1, func=SQRT)
                # rs1 materialized dense bf16 (stride-0 reads are free on ACT)
                nc.scalar.activation(out=RS1d, in_=bcast(RS1, R), func=COPY)

                # w = v - (dot * is1) d1   (f32 rejection; W sub on Pool)
                nc.vector.tensor_mul(G, SD[:, :, 1], IS1)
                nc.vector.tensor_mul(T1, D1, bcast(G, R))
                nc.vector.tensor_sub(W, V, T1)

                # rs2 = rsqrt(|w|^2), materialized dense bf16
                nc.scalar.activation(out=SQ2, in_=W, func=SQUARE)
                nc.vector.reduce_sum(out=S2, in_=SQ2, axis=AX)
                nc.vector.reciprocal_approx_fast(out=IS2, in_=S2)
                nc.scalar.activation(out=RS2, in_=IS2, func=SQRT)
                nc.scalar.activation(out=RS2d, in_=bcast(RS2, R), func=COPY)
                nc.scalar.activation(out=Wb, in_=W, func=COPY)
                return st

            def tail(st):
                R = st["R"]
                E1 = v3.tile([P, R, 3], bf16, tag="e1", name="e1")
                E2 = v3.tile([P, R, 3], bf16, tag="e2", name="e2")
                SA = v3.tile([P, R, 3], bf16, tag="sa", name="sa")
                SB = v3.tile([P, R, 3], bf16, tag="sb", name="sb")
                TN = v3.tile([P, R, 3], bf16, tag="tn", name="tn")
                TO = v3.tile([P, R, 3], bf16, tag="to", name="to")
                OUT = io.tile([P, 3, R, 3], bf16, tag="OUT", name="OUT")
                CAb = st["CAb"]

                # e1/e2 as dense bf16 x bf16 products (2x mode)
                nc.vector.tensor_mul(E1, st["D1b"], st["RS1d"])
                nc.vector.tensor_mul(E2, st["Wb"], st["RS2d"])
                # atom-major OUT: [:,0]=N, [:,1]=C, [:,2]=O.
                # TS prescale runs 4x, bf16 TT add runs 2x.
                nc.vector.tensor_scalar_mul(out=SA, in0=E1, scalar1=1.526)
                nc.vector.tensor_add(OUT[:, 1], SA, CAb)
                nc.vector.tensor_scalar_mul(out=SB, in0=E2, scalar1=1.363)
                nc.vector.tensor_add(TN, SB, CAb)
                nc.vector.tensor_scalar_mul(out=SA, in0=E1, scalar1=-0.525)
                nc.vector.tensor_add(OUT[:, 0], SA, TN)
                nc.vector.tensor_scalar_mul(out=SB, in0=E2, scalar1=-1.062)
                nc.vector.tensor_add(TO, SB, CAb)
                nc.vector.tensor_scalar_mul(out=SA, in0=E1, scalar1=2.153)
                nc.vector.tensor_add(OUT[:, 2], SA, TO)
                nc.scalar.dma_start(
                    out=Y[:, st["off"]:st["off"] + P * st["R"], :].rearrange(
                        "a (p r) c -> p a r c", p=P),
                    in_=OUT)

            offs = []
            o = 0
            for R in TILE_SIZES:
                offs.append(o)
                o += P * R
            assert o == N_CORE
            sts = [head(offs[i], R) for i, R in enumerate(TILE_SIZES)]
            for st in sts:
                tail(st)
    nc.finalize()
    return nc


def _get_nc():
    global _NC
    if _NC is None:
        _NC = _build_nc()
    return _NC


def make_in_maps(X):
    """Pack (N, CA, C) as contiguous [N_CORE, 9] f32 shards per core."""
    X = np.asarray(X, dtype=np.float32)
    A = np.ascontiguousarray(X[:, 0:3, :]).reshape(N_TOTAL, CIN)
    shards = A.reshape(N_CORES, N_CORE, CIN)
    return [{"X": shards[c]} for c in range(N_CORES)]


def assemble(X, results):
    """Stitch device outputs (atom-major bf16 [3, rows, 3]) into the
    full [N, 14, 3] f32 array."""
    out = np.array(X, dtype=np.float32, copy=True)
    Ys = [np.asarray(results[c]["Y"]).astype(np.float32)
          for c in range(N_CORES)]
    Yall = np.concatenate(Ys, axis=1)  # [3, N_TOTAL, 3]
    out[:, 0, :] = Yall[0]
    out[:, 2, :] = Yall[1]
    out[:, 3, :] = Yall[2]
    return out


def kernel(X, batch_ids=None, max_len=None, **_unused):
    from concourse.bass_utils import run_bass_kernel_spmd

    X = np.asarray(X, dtype=np.float32)
    assert X.shape == (N_TOTAL, 14, 3), X.shape
    nc = _get_nc()
    res = run_bass_kernel_spmd(nc, make_in_maps(X), list(range(N_CORES))).results
    return assemble(X, res)


# revision 8
# speedup vs baseline: 1.1996x; 1.1996x over previous
"""Trainium2 Bass kernel for nn_BackboneModel (backbone frame rebuild).

The reference scatters rows into a padded [B, L, 14, 3] block, builds
Gram-Schmidt rigid frames from (N, CA, C), places ideal N/CA/C/O atoms,
and gathers the valid rows back.  Scatter followed by gather at the same
(batch_id, pos) indices is an identity permutation over the valid rows,
so the whole model is a pure per-row function of X[i]:

    e1 = normalize(C - CA)                      (normalize: v * rsqrt(|v|^2 + eps^2))
    e2 = normalize((N - CA) - ((N - CA).e1) e1)
    out[0] = -0.525*e1 + 1.363*e2 + CA          (N)
    out[1] = CA                                 (CA, passthrough)
    out[2] =  1.526*e1            + CA          (C)
    out[3] =  2.153*e1 - 1.062*e2 + CA          (O)
    out[4:14] = X[4:14]                         (passthrough)

Only atoms 0..2 (N, CA, C) feed any arithmetic, and only atoms 0, 2, 3
get new values.  The device reads a packed [rows, 9] f32 block (N, CA,
C) and writes a packed atom-major [3, rows, 3] bf16 block (outN, outC,
outO); CA and atoms 4..13 are stitched in on the host during unshard
(pure data movement).  Correctness gate is rel-L2 < 2e-2; bf16 rounding
of final coordinates contributes ~1e-3.

Numerics: the Gram-Schmidt rejection w = v - (v.e1)e1 amplifies error in
the projection scalar g by ~|v|/|w|, so g = (v.d1)/|d1|^2 uses the DVE
reciprocal-approx (18 bits) and w is formed in f32.  The unit scalings
rs1/rs2 only scale outputs, so the ACT table Sqrt is fine there.  The
reference's +eps^2 regularizer is dropped (inputs are randn; |d1|^2 is
never near eps^2 = 1e-6, relative effect ~1e-8).

Sharding: data-parallel, 8 equal contiguous row chunks of 98304 rows.
"""

import numpy as np

N_CORES = 8
N_TOTAL = 786432
N_CORE = N_TOTAL // N_CORES      # 98304 rows per core
P = 128                          # SBUF partitions
ROWS_PER_PART = N_CORE // P      # 768 rows per partition per core
TILE_SIZES = [64, 224, 256, 224]
CIN = 9                          # N, CA, C xyz (f32)
EPS2 = 1e-6                      # unused on device; kept for reference

_NC = None


def _build_nc():
    import concourse.bacc as bacc
    import concourse.tile as tile
    from concourse import mybir

    f32 = mybir.dt.float32
    bf16 = mybir.dt.bfloat16
    MUL = mybir.AluOpType.mult
    ADD = mybir.AluOpType.add
    AX = mybir.AxisListType.X
    SQRT = mybir.ActivationFunctionType.Sqrt
    SQUARE = mybir.ActivationFunctionType.Square
    COPY = mybir.ActivationFunctionType.Copy

    nc = bacc.Bacc()
    X = nc.declare_dram_parameter("X", [N_CORE, CIN], f32, isOutput=False)
    Y = nc.declare_dram_parameter("Y", [3, N_CORE, 3], bf16, isOutput=True)

    def bcast(s, r):  # [P, r] per-row scalar -> [P, r, 3]
        return s[:, :, None].broadcast_to([P, r, 3])

    with tile.TileContext(nc) as tc:
        with tc.tile_pool(name="io", bufs=4) as io, \
             tc.tile_pool(name="v3", bufs=3) as v3, \
             tc.tile_pool(name="sc", bufs=3) as sc:

            def head(row_off, R):
                st = {"R": R, "off": row_off}
                T = st["T"] = io.tile([P, R, CIN], f32, tag="T", name="T")
                nc.sync.dma_start(
                    out=T,
                    in_=X[row_off:row_off + P * R, :].rearrange(
                        "(p r) c -> p r c", p=P))
                Na = T[:, :, 0:3]
                CAa = T[:, :, 3:6]
                Ca = T[:, :, 6:9]

                D1 = v3.tile([P, R, 3], f32, tag="d1", name="d1")
                V = v3.tile([P, R, 3], f32, tag="v", name="v")
                QQ = v3.tile([P, R, 6], f32, tag="qq", name="qq")
                T1 = v3.tile([P, R, 3], f32, tag="t1", name="t1")
                W = v3.tile([P, R, 3], f32, tag="w", name="w")
                SQ2 = v3.tile([P, R, 3], f32, tag="sq2", name="sq2")
                CAb = st["CAb"] = v3.tile([P, R, 3], bf16, tag="cab",
                                          name="cab")
                D1b = st["D1b"] = v3.tile([P, R, 3], bf16, tag="d1b",
                                          name="d1b")
                Wb = st["Wb"] = v3.tile([P, R, 3], bf16, tag="wb", name="wb")
                RS1d = st["RS1d"] = v3.tile([P, R, 3], bf16, tag="rs1d",
                                            name="rs1d")
                RS2d = st["RS2d"] = v3.tile([P, R, 3], bf16, tag="rs2d",
                                            name="rs2d")
                SD = sc.tile([P, R, 2], f32, tag="sd", name="sd")
                IS1 = sc.tile([P, R], f32, tag="is1", name="is1")
                RS1 = sc.tile([P, R], f32, tag="rs1", name="rs1")
                G = sc.tile([P, R], f32, tag="g", name="g")
                S2 = sc.tile([P, R], f32, tag="s2", name="s2")
                IS2 = sc.tile([P, R], f32, tag="is2", name="is2")
                RS2 = sc.tile([P, R], f32, tag="rs2", name="rs2")

                # d1 = C - CA, v = N - CA  (Pool engine)
                nc.gpsimd.tensor_sub(D1, Ca, CAa)
                nc.gpsimd.tensor_sub(V, Na, CAa)
                # bf16 shadows for the 2x tail (ACT)
                nc.scalar.activation(out=CAb, in_=CAa, func=COPY)
                nc.scalar.activation(out=D1b, in_=D1, func=COPY)

                # fused |d1|^2 and v.d1: QQ = [d1^2 | v*d1], one reduce
                nc.scalar.activation(out=QQ[:, :, 0:3], in_=D1, func=SQUARE)
                nc.vector.tensor_mul(QQ[:, :, 3:6], V, D1)
                nc.vector.reduce_sum(
                    out=SD, in_=QQ.rearrange("p r (a c) -> p r a c", a=2),
                    axis=AX)
                nc.vector.reciprocal_approx_fast(out=IS1, in_=SD[:, :, 0])
                nc.scalar.activation(out=RS1, in_=IS1, func=SQRT)
                # rs1 materialized dense bf16 (stride-0 reads are free on ACT)
                nc.scalar.activation(out=RS1d, in_=bcast(RS1, R), func=COPY)

                # w = v - (dot * is1) d1   (f32 rejection; W sub on Pool)
                nc.vector.tensor_mul(G, SD[:, :, 1], IS1)
                nc.vector.tensor_mul(T1, D1, bcast(G, R))
                nc.vector.tensor_sub(W, V, T1)

                # rs2 = rsqrt(|w|^2), materialized dense bf16
                nc.scalar.activation(out=SQ2, in_=W, func=SQUARE)
                nc.vector.reduce_sum(out=S2, in_=SQ2, axis=AX)
                nc.vector.reciprocal_approx_fast(out=IS2, in_=S2)
                nc.scalar.activation(out=RS2, in_=IS2, func=SQRT)
                nc.scalar.activation(out=RS2d, in_=bcast(RS2, R), func=COPY)
                nc.scalar.activation(out=Wb, in_=W, func=COPY)
                return st

            def tail(st):
                R = st["R"]
                E1 = v3.tile([P, R, 3], bf16, tag="e1", name="e1")
                E2 = v3.tile([P, R, 3], bf16, tag="e2", name="e2")
                SA = v3.tile([P, R, 3], bf16, tag="sa", name="sa")
                SB = v3.tile([P, R, 3], bf16, tag="sb", name="sb")
                TN = v3.tile([P, R, 3], bf16, tag="tn", name="tn")
                TO = v3.tile([P, R, 3], bf16, tag="to", name="to")
                OUT = io.tile([P, 3, R, 3], bf16, tag="OUT", name="OUT")
                CAb = st["CAb"]

                # e1/e2 as dense bf16 x bf16 products (2x mode)
                nc.vector.tensor_mul(E1, st["D1b"], st["RS1d"])
                nc.vector.tensor_mul(E2, st["Wb"], st["RS2d"])
                # atom-major OUT: [:,0]=N, [:,1]=C, [:,2]=O.
                # TS prescale runs 4x, bf16 TT add runs 2x.
                nc.vector.tensor_scalar_mul(out=SA, in0=E1, scalar1=1.526)
                nc.vector.tensor_add(OUT[:, 1], SA, CAb)
                nc.vector.tensor_scalar_mul(out=SB, in0=E2, scalar1=1.363)
                nc.vector.tensor_add(TN, SB, CAb)
                nc.vector.tensor_scalar_mul(out=SA, in0=E1, scalar1=-0.525)
                nc.vector.tensor_add(OUT[:, 0], SA, TN)
                nc.vector.tensor_scalar_mul(out=SB, in0=E2, scalar1=-1.062)
                nc.vector.tensor_add(TO, SB, CAb)
                nc.vector.tensor_scalar_mul(out=SA, in0=E1, scalar1=2.153)
                nc.vector.tensor_add(OUT[:, 2], SA, TO)
                nc.scalar.dma_start(
                    out=Y[:, st["off"]:st["off"] + P * st["R"], :].rearrange(
                        "a (p r) c -> p a r c", p=P),
                    in_=OUT)

            offs = []
            o = 0
            for R in TILE_SIZES:
                offs.append(o)
                o += P * R
            assert o == N_CORE
            sts = [head(offs[i], R) for i, R in enumerate(TILE_SIZES)]
            for st in sts:
                tail(st)
    nc.finalize()
    return nc


def _get_nc():
    global _NC
    if _NC is None:
        _NC = _build_nc()
    return _NC


def make_in_maps(X):
    """Pack (N, CA, C) as contiguous [N_CORE, 9] f32 shards per core."""
    X = np.asarray(X, dtype=np.float32)
    A = np.ascontiguousarray(X[:, 0:3, :]).reshape(N_TOTAL, CIN)
    shards = A.reshape(N_CORES, N_CORE, CIN)
    return [{"X": shards[c]} for c in range(N_CORES)]


def assemble(X, results):
    """Stitch device outputs (atom-major bf16 [3, rows, 3]) into the
    full [N, 14, 3] f32 array."""
    out = np.array(X, dtype=np.float32, copy=True)
    Ys = [np.asarray(results[c]["Y"]).astype(np.float32)
          for c in range(N_CORES)]
    Yall = np.concatenate(Ys, axis=1)  # [3, N_TOTAL, 3]
    out[:, 0, :] = Yall[0]
    out[:, 2, :] = Yall[1]
    out[:, 3, :] = Yall[2]
    return out


def kernel(X, batch_ids=None, max_len=None, **_unused):
    from concourse.bass_utils import run_bass_kernel_spmd

    X = np.asarray(X, dtype=np.float32)
    assert X.shape == (N_TOTAL, 14, 3), X.shape
    nc = _get_nc()
    res = run_bass_kernel_spmd(nc, make_in_maps(X), list(range(N_CORES))).results
    return assemble(X, res)
